# revision 59
# baseline (speedup 1.0000x reference)
"""Trainium2 Bass kernel for complex-valued spatial-reduction attention.

x: [B=4, N=2304, C=512] complex64 (re/im f32 planes), H=W=48, 8 heads,
head_dim 64, sr_ratio 2 -> Nk=576.

Sharding: 8 cores = 4 batches x 2 head-groups (4 heads each). Each core:
sr-conv over full C, complex LayerNorm, q/k/v for its heads,
softmax(|q.k^T|) attention, attn @ v, partial output projection.
Host sums the two partials per batch and adds bproj.

Precision: softmax logits reach |a| ~ 37, so the conv -> LN-stats ->
q/kv -> scores chain runs in f32r matmuls (f32-equivalent precision at
bf16-class speed for free dims >= 256, measured on HW). E/v/attn@v/proj
use bf16.

Structure (one pass, phases overlap via per-engine emission order):
- conv decomposed by patch position, accumulated in SBUF so the 12.6MB
  weight streams exactly once; LN stats batched across all 5 row chunks
- q^T stays resident in SBUF (no DRAM round-trip); wq/wk cached
- attention runs a 3-iteration-deep software pipeline with a two-stage
  normalization tail and the output projection folded in, so the softmax
  chain's latency never blocks the PE
- activation-table trick: phase 1 runs entirely out of sqrt_and_others,
  the attention phase out of natural_log_exp_and_others (2 table loads
  total instead of 94)
"""

import contextlib

import numpy as np
import ml_dtypes

import concourse.bass as bass
import concourse.mybir as mybir
import concourse.tile as tile
from concourse import bacc
from concourse.masks import make_identity

BF16 = mybir.dt.bfloat16
F16 = mybir.dt.float16
F32 = mybir.dt.float32
F32R = mybir.dt.float32r
AF = mybir.ActivationFunctionType
ALU = mybir.AluOpType

B, N, C, HEADS, HD, SR = 4, 2304, 512, 8, 64, 2
NK = 576
HR = 24
EPS = 1e-5
SCALE = HD ** -0.5  # folded into Wk host-side

K_CHUNKS = [(0, 128), (128, 128), (256, 128), (384, 128), (512, 64)]
Q_CHUNKS = [(0, 512), (512, 512), (1024, 512), (1536, 512), (2048, 256)]


def _r(ap):
    return ap.bitcast(F32R)


def _trim_act_tables(arch):
    """Steer the act-table placement pass to sets that cover whole phases.

    The greedy inserter picks the first table containing the missing
    function: Ln -> natural_log (5), Exp -> exp_and_others (0), so the
    attention loop ping-pongs 0<->5 (~94 loads, ~1.3us each).  Trimming
    the cached table metadata makes the first match for Ln AND Exp be
    natural_log_exp_and_others (6), and Sign resolve to sqrt_and_others
    (3) alongside phase 1's Sqrt.  The real HW tables are unchanged and
    genuinely contain every function executed under them (6 = union of
    0 and 5), so the compiled program remains valid -- it just loads 2
    tables instead of 94.
    """
    from concourse.hw_specs import get_activation_tables

    tabs = get_activation_tables(arch)
    names = list(tabs)
    tabs[names[0]].discard(AF.Sign)
    tabs[names[1]].discard(AF.Sign)
    tabs[names[2]].discard(AF.Sign)
    tabs[names[5]].discard(AF.Ln)
    tabs[names[0]].discard(AF.Exp)


def build_nc():
    nc = bacc.Bacc("TRN2", target_bir_lowering=False, debug=False, num_devices=8)
    _trim_act_tables(nc.m.arch)

    xT_d = nc.dram_tensor("xT", [2, C, N], F32R, kind="ExternalInput")
    xP_d = nc.dram_tensor("xP", [2, 4 * C, NK], F32R, kind="ExternalInput")
    wc_d = nc.dram_tensor("wc", [3, 4 * C, C], F32R, kind="ExternalInput")
    srb_d = nc.dram_tensor("srb", [2, C], F32R, kind="ExternalInput")
    ones_d = nc.dram_tensor("ones", [1, 512], F32R, kind="ExternalInput")
    wq_d = nc.dram_tensor("wq", [3, C, 256], F32R, kind="ExternalInput")
    wk_d = nc.dram_tensor("wk", [3, C, 256], F32R, kind="ExternalInput")
    wv_d = nc.dram_tensor("wv", [3, C, 256], F32R, kind="ExternalInput")
    wp_d = nc.dram_tensor("wp", [3, 256, C], BF16, kind="ExternalInput")
    bkv_d = nc.dram_tensor("bkv", [2, 2, 256], F32R, kind="ExternalInput")
    outT_d = nc.dram_tensor("outT", [2, C, N], F16, kind="ExternalOutput")

    with tile.TileContext(nc) as tc:
        _body(nc, tc, xT_d, xP_d, wc_d, srb_d, ones_d, wq_d, wk_d, wv_d, wp_d,
              bkv_d, outT_d)

    nc.compile()
    return nc


def _ln_sums(nc, work, st5, re_sb, im_sb, sz, mc):
    """Row sums/moments for one [sz, C] chunk -> column mc of the [128,5]
    stat matrices."""
    sum_r5, sum_i5, sxx5, sii5, sxi5 = st5
    nc.vector.tensor_reduce(sum_r5[:sz, mc:mc + 1], re_sb[:sz],
                            mybir.AxisListType.X, ALU.add)
    nc.vector.tensor_reduce(sum_i5[:sz, mc:mc + 1], im_sb[:sz],
                            mybir.AxisListType.X, ALU.add)
    junk = work.tile([128, C], F32, tag="ln_junk", bufs=1)
    nc.gpsimd.tensor_mul(junk[:sz], re_sb[:sz], re_sb[:sz])
    nc.vector.tensor_reduce(sxx5[:sz, mc:mc + 1], junk[:sz],
                            mybir.AxisListType.X, ALU.add)
    junk2 = work.tile([128, C], F32, tag="ln_junk2", bufs=1)
    nc.gpsimd.tensor_mul(junk2[:sz], im_sb[:sz], im_sb[:sz])
    nc.vector.tensor_reduce(sii5[:sz, mc:mc + 1], junk2[:sz],
                            mybir.AxisListType.X, ALU.add)
    junk3 = work.tile([128, C], F32, tag="ln_junk3", bufs=1)
    nc.gpsimd.tensor_mul(junk3[:sz], re_sb[:sz], im_sb[:sz])
    nc.vector.tensor_reduce(sxi5[:sz, mc:mc + 1], junk3[:sz],
                            mybir.AxisListType.X, ALU.add)


def _ln_stats(nc, stats, st5):
    """Complex-LN statistics for all 5 chunks at once on [128, 5] tiles.
    Unwritten rows of the short chunk flow garbage through; their columns
    are never read back."""
    sum_r, sum_i, sxx, sii, sxi = st5
    inv_c = 1.0 / C
    W = 5

    def T(tag):
        return stats.tile([128, W], F32, tag=tag, name=tag)

    mr, mi = T("mr"), T("mi")
    nc.gpsimd.tensor_scalar_mul(mr[:], sum_r[:], inv_c)
    nc.gpsimd.tensor_scalar_mul(mi[:], sum_i[:], inv_c)
    vre, vim, tA, tB = T("vre"), T("vim"), T("tA"), T("tB")
    nc.gpsimd.tensor_sub(tA[:], sxx[:], sii[:])
    nc.gpsimd.tensor_scalar_mul(tA[:], tA[:], inv_c)
    nc.gpsimd.tensor_mul(vre[:], mr[:], mr[:])
    nc.gpsimd.tensor_mul(tB[:], mi[:], mi[:])
    nc.gpsimd.tensor_sub(vre[:], vre[:], tB[:])
    nc.gpsimd.tensor_sub(vre[:], tA[:], vre[:])
    nc.gpsimd.tensor_scalar_add(vre[:], vre[:], EPS)
    nc.gpsimd.tensor_mul(tB[:], mr[:], mi[:])
    nc.gpsimd.tensor_scalar_mul(tB[:], tB[:], 2.0)
    nc.gpsimd.tensor_scalar_mul(vim[:], sxi[:], 2.0 * inv_c)
    nc.gpsimd.tensor_sub(vim[:], vim[:], tB[:])
    r2 = T("r2")
    nc.gpsimd.tensor_mul(r2[:], vre[:], vre[:])
    nc.gpsimd.tensor_mul(tB[:], vim[:], vim[:])
    nc.gpsimd.tensor_add(r2[:], r2[:], tB[:])

    def _sqrt_newton(out, x, sc):
        # y0 = LUT sqrt(sc*x); y1 = 0.5*(y0 + sc*x/y0)  (one Newton step)
        y0 = T("nw_y0")
        nc.scalar.activation(y0[:], x[:], AF.Sqrt, scale=sc)
        yr = T("nw_yr")
        nc.vector.tensor_scalar_add(y0[:], y0[:], 1e-30)
        nc.vector.reciprocal(yr[:], y0[:])
        nc.vector.tensor_mul(yr[:], yr[:], x[:])
        if sc != 1.0:
            nc.vector.tensor_scalar_mul(yr[:], yr[:], sc)
        nc.vector.tensor_add(out[:], y0[:], yr[:])
        nc.vector.tensor_scalar_mul(out[:], out[:], 0.5)

    rr = T("rr")
    _sqrt_newton(rr, r2, 1.0)
    srt, sia = T("srt"), T("sia")
    nc.vector.tensor_add(tA[:], rr[:], vre[:])
    _sqrt_newton(srt, tA, 0.5)
    nc.vector.tensor_sub(tA[:], rr[:], vre[:])
    _sqrt_newton(sia, tA, 0.5)
    sgn = T("sgn")
    nc.scalar.activation(sgn[:], vim[:], AF.Sign)
    nc.vector.tensor_mul(sia[:], sia[:], sgn[:])
    rin = T("rin")
    nc.vector.reciprocal(rin[:], rr[:])
    wr, wn = T("wr"), T("wn")  # wn = -w_im
    nc.vector.tensor_mul(wr[:], srt[:], rin[:])
    nc.vector.tensor_mul(wn[:], sia[:], rin[:])
    return mr, mi, wr, wn


def _ln_norm(nc, work, lnst, re_sb, im_sb, sz, mc):
    """Apply the complex LN transform for chunk mc -> (xnr, xni)."""
    mr5, mi5, wr5, wn5 = lnst
    mr = mr5[:, mc:mc + 1]
    mi = mi5[:, mc:mc + 1]
    wr = wr5[:, mc:mc + 1]
    wn = wn5[:, mc:mc + 1]
    aT = work.tile([128, C], F32, tag="ln_junk", bufs=1)
    bT = work.tile([128, C], F32, tag="ln_junk2", bufs=1)
    xnr = work.tile([128, C], F32, tag="ln_xnr", bufs=1)
    xni = work.tile([128, C], F32, tag="ln_xni", bufs=1)
    nc.gpsimd.tensor_scalar(aT[:sz], re_sb[:sz], mr[:sz], wr[:sz],
                            ALU.subtract, ALU.mult)
    nc.gpsimd.tensor_scalar(bT[:sz], im_sb[:sz], mi[:sz], wn[:sz],
                            ALU.subtract, ALU.mult)
    nc.gpsimd.tensor_add(xnr[:sz], aT[:sz], bT[:sz])
    nc.gpsimd.tensor_scalar(aT[:sz], re_sb[:sz], mr[:sz], wn[:sz],
                            ALU.subtract, ALU.mult)
    nc.gpsimd.tensor_scalar(bT[:sz], im_sb[:sz], mi[:sz], wr[:sz],
                            ALU.subtract, ALU.mult)
    nc.gpsimd.tensor_sub(xni[:sz], bT[:sz], aT[:sz])
    return xnr, xni


def _body(nc, tc, xT_d, xP_d, wc_d, srb_d, ones_d, wq_d, wk_d, wv_d, wp_d,
          bkv_d, outT_d):
    ctx = contextlib.ExitStack()
    consts = ctx.enter_context(tc.tile_pool(name="consts", bufs=1))
    big = ctx.enter_context(tc.tile_pool(name="big", bufs=1))
    work = ctx.enter_context(tc.tile_pool(name="work", bufs=2))
    stats = ctx.enter_context(tc.tile_pool(name="stats", bufs=2))
    psum = ctx.enter_context(tc.tile_pool(name="psum", bufs=8, space="PSUM"))
    # phase-1/2-only pools -- closed before the attention pools open so the
    # SBUF is time-shared
    p1ctx = contextlib.ExitStack()
    cv = p1ctx.enter_context(tc.tile_pool(name="cv", bufs=1))
    qs = p1ctx.enter_context(tc.tile_pool(name="qs", bufs=2))
    ws = p1ctx.enter_context(tc.tile_pool(name="ws", bufs=2))

    # ---- constants ----
    ident = consts.tile([128, 128], F32, tag="ident")
    make_identity(nc, ident)
    ones_col = consts.tile([128, 1], BF16, tag="ones_col")
    nc.vector.memset(ones_col, 1.0)
    ones_row = consts.tile([1, 512], F32R, tag="ones_row")
    nbias = consts.tile([128, 1], F32, tag="nbias")
    nc.vector.memset(nbias, -50.0)
    ones2 = consts.tile([33, 128], F32, tag="ones2")
    nc.vector.memset(ones2, 1.0)

    srb_re = cv.tile([1, C], F32R, tag="srb_re")
    srb_im = cv.tile([1, C], F32R, tag="srb_im")
    bk_re = cv.tile([1, 256], F32R, tag="bk_re")
    bk_im = cv.tile([1, 256], F32R, tag="bk_im")
    bv_re = cv.tile([1, 256], F32R, tag="bv_re")
    bv_im = cv.tile([1, 256], F32R, tag="bv_im")

    # ---- persistent SBUF ----
    xnTr = big.tile([128, 4, NK], F32R, tag="xnTr")
    xnTi = big.tile([128, 4, NK], F32R, tag="xnTi")
    kTr = big.tile([128, 2, NK], F32R, tag="kTr")
    kTi = big.tile([128, 2, NK], F32R, tag="kTi")
    kTin = big.tile([128, 2, NK], F32R, tag="kTin")
    vpk = big.tile([128, 5, 4, 128], BF16, tag="vpk")
    # q^T lives in SBUF for the whole kernel: [plane, half, n]
    qT_sb = big.tile([128, 2, 2, N], F32R, tag="qT_sb")
    # wq cached across the 5 q-chunks: [cj, plane, out]
    wq_sb = big.tile([128, 4, 3, 256], F32R, tag="wq_sb")

    xT_v = [xT_d[pl].rearrange("(j p) t -> p j t", p=128) for pl in (0, 1)]

    # =====================================================================
    # Phase 1: sr-conv, decomposed by patch position so wc streams ONCE.
    # Accumulate the 4 positions in SBUF (acc_re/acc_im), LayerNorm reads
    # the accumulators directly.  LN chunks are interleaved with q-proj
    # chunks so the PE has matmul work while LN runs on DVE/ACT.
    # =====================================================================
    MC = [(0, 120), (120, 120), (240, 120), (360, 120), (480, 96)]
    acc_re = cv.tile([128, 5, C], F32, tag="acc_re")
    acc_im = cv.tile([128, 5, C], F32, tag="acc_im")

    for pos in range(4):
        if pos == 1:
            # deferred constant loads: nothing upstream needs them until
            # the pos-3 bias matmuls / kproj, so the pos-0 conv tiles win
            # the DMA queues
            nc.sync.dma_start(ones_row[:], ones_d[:])
            nc.sync.dma_start(srb_re[:], srb_d[0:1, :])
            nc.sync.dma_start(srb_im[:], srb_d[1:2, :])
            nc.sync.dma_start(bk_re[:], bkv_d[0:1, 0, :])
            nc.sync.dma_start(bv_re[:], bkv_d[0:1, 1, :])
            nc.sync.dma_start(bk_im[:], bkv_d[1:2, 0, :])
            nc.sync.dma_start(bv_im[:], bkv_d[1:2, 1, :])
        wts = []
        xps = []
        for j in range(4):
            r0 = 512 * pos + 128 * j
            w = cv.tile([128, 3, C], F32R, tag="wpos", bufs=4)
            nc.gpsimd.dma_start(w[:, 0], wc_d[0, r0:r0 + 128, :])
            nc.sync.dma_start(w[:, 1], wc_d[1, r0:r0 + 128, :])
            nc.gpsimd.dma_start(w[:, 2], wc_d[2, r0:r0 + 128, :])
            xp = cv.tile([128, 2, NK], F32R, tag="xpos", bufs=4)
            nc.sync.dma_start(xp[:, 0], xP_d[0, r0:r0 + 128, :])
            nc.gpsimd.dma_start(xp[:, 1], xP_d[1, r0:r0 + 128, :])
            wts.append(w)
            xps.append(xp)
        for (t0, sz) in MC:
            cre = psum.tile([128, C], F32, tag="bank")
            cim = psum.tile([128, C], F32, tag="bank")
            for j in range(4):
                w, xp = wts[j], xps[j]
                pat_r = xp[:, 0, t0:t0 + sz]
                pat_i = xp[:, 1, t0:t0 + sz]
                st = j == 0
                nc.tensor.matmul(cre[:sz, :], pat_r, w[:, 0], start=st,
                                 stop=False)
                nc.tensor.matmul(cim[:sz, :], pat_r, w[:, 1], start=st,
                                 stop=False)
                nc.tensor.matmul(cre[:sz, :], pat_i, w[:, 2], start=False,
                                 stop=False)
                nc.tensor.matmul(cim[:sz, :], pat_i, w[:, 0], start=False,
                                 stop=j == 3 and pos != 3)
            if pos == 3:
                nc.tensor.matmul(cre[:sz, :], _r(ones_row[:, :sz]),
                                 _r(srb_re[:]), start=False, stop=True)
                nc.tensor.matmul(cim[:sz, :], _r(ones_row[:, :sz]),
                                 _r(srb_im[:]), start=False, stop=True)
            mc = MC.index((t0, sz))
            if pos == 0:
                nc.scalar.copy(acc_re[:sz, mc], cre[:sz, :])
                nc.scalar.copy(acc_im[:sz, mc], cim[:sz, :])
            else:
                nc.vector.tensor_add(acc_re[:sz, mc], cre[:sz, :],
                                     acc_re[:sz, mc])
                nc.vector.tensor_add(acc_im[:sz, mc], cim[:sz, :],
                                     acc_im[:sz, mc])

    for s_ in range(3):
        nc.sync.dma_start(wq_sb[:, :, s_, :],
                          wq_d[s_].rearrange("(j p) n -> p j n", p=128))

    st5 = tuple(stats.tile([128, 5], F32, tag=t_, bufs=1, name=t_)
                for t_ in ("sum_r5", "sum_i5", "sxx5", "sii5", "sxi5"))
    lnst = [None]

    def emit_ln(mc):
        t0, sz = MC[mc]
        xnr, xni = _ln_norm(nc, work, lnst[0], acc_re[:, mc],
                            acc_im[:, mc], sz, mc)
        for cj in range(4):
            for srcT, dst in ((xnr, xnTr), (xni, xnTi)):
                pt = psum.tile([128, 128], F32, tag="bank")
                nc.tensor.transpose(pt[:, :sz],
                                    srcT[:sz, 128 * cj:128 * (cj + 1)],
                                    ident[:sz, :sz])
                nc.scalar.copy(dst[:, cj, t0:t0 + sz], pt[:, :sz])

    # =====================================================================
    # Phase 1b: q-projection (f32r), x^T streamed, q^T kept in SBUF
    # =====================================================================
    def emit_qproj(q0, nq):
        prs = []
        for half in range(2):
            prs.append((psum.tile([128, 512], F32, tag="bank", name=f"qpr{half}"),
                        psum.tile([128, 512], F32, tag="bank", name=f"qpi{half}")))
        for cj in range(4):
            xq_r = qs.tile([128, 512], F32R, tag="xq_r", bufs=2)
            xq_i = qs.tile([128, 512], F32R, tag="xq_i", bufs=2)
            nc.gpsimd.dma_start(xq_r[:, :nq], xT_v[0][:, cj, q0:q0 + nq])
            nc.sync.dma_start(xq_i[:, :nq], xT_v[1][:, cj, q0:q0 + nq])
            st = cj == 0
            sp = cj == 3
            for half in range(2):
                hs = slice(128 * half, 128 * (half + 1))
                pr, pi = prs[half]
                nc.tensor.matmul(pr[:, :nq], wq_sb[:, cj, 0, hs], _r(xq_r[:, :nq]),
                                 start=st, stop=False)
                nc.tensor.matmul(pr[:, :nq], wq_sb[:, cj, 2, hs], _r(xq_i[:, :nq]),
                                 start=False, stop=sp)
                nc.tensor.matmul(pi[:, :nq], wq_sb[:, cj, 1, hs], _r(xq_r[:, :nq]),
                                 start=st, stop=False)
                nc.tensor.matmul(pi[:, :nq], wq_sb[:, cj, 0, hs], _r(xq_i[:, :nq]),
                                 start=False, stop=sp)
        for half in range(2):
            pr, pi = prs[half]
            nc.scalar.copy(qT_sb[:, 0, half, q0:q0 + nq], pr[:, :nq])
            nc.scalar.copy(qT_sb[:, 1, half, q0:q0 + nq], pi[:, :nq])

    # =====================================================================
    # Phase 2: k^T (two NK-halves of 288, so every matmul free dim >= 256)
    # and v projections (f32r).  Emitted interleaved with LN chunks.
    # =====================================================================
    wk_sb = cv.tile([128, 4, 3, 256], F32R, tag="wk_sb")

    def emit_kproj(nkh):
        n0, nn = 288 * nkh, 288
        for half in range(2):
            hs = slice(128 * half, 128 * (half + 1))
            pr = psum.tile([128, 512], F32, tag="bank")
            pi = psum.tile([128, 512], F32, tag="bank")
            for cj in range(4):
                st = cj == 0
                nc.tensor.matmul(pr[:, :nn], wk_sb[:, cj, 0, hs],
                                 _r(xnTr[:, cj, n0:n0 + nn]), start=st,
                                 stop=False)
                nc.tensor.matmul(pr[:, :nn], wk_sb[:, cj, 2, hs],
                                 _r(xnTi[:, cj, n0:n0 + nn]), start=False,
                                 stop=False)
                nc.tensor.matmul(pi[:, :nn], wk_sb[:, cj, 1, hs],
                                 _r(xnTr[:, cj, n0:n0 + nn]), start=st,
                                 stop=False)
                nc.tensor.matmul(pi[:, :nn], wk_sb[:, cj, 0, hs],
                                 _r(xnTi[:, cj, n0:n0 + nn]), start=False,
                                 stop=False)
            nc.tensor.matmul(pr[:, :nn], _r(bk_re[:, hs]), _r(ones_row[:, :nn]),
                             start=False, stop=True)
            nc.tensor.matmul(pi[:, :nn], _r(bk_im[:, hs]), _r(ones_row[:, :nn]),
                             start=False, stop=True)
            nc.scalar.copy(kTr[:, half, n0:n0 + nn], pr[:, :nn])
            nc.scalar.copy(kTi[:, half, n0:n0 + nn], pi[:, :nn])
            nc.scalar.mul(kTin[:, half, n0:n0 + nn], pi[:, :nn], -1.0)

    def emit_vproj(kcg):
        pps = {}
        for kc in kcg:
            pps[kc] = (psum.tile([128, 512], F32, tag="bank", name=f"vpr{kc}"),
                       psum.tile([128, 512], F32, tag="bank", name=f"vpi{kc}"))
        for cj in range(4):
            wv_r = ws.tile([128, 256], F32R, tag="w_r")
            wv_i = ws.tile([128, 256], F32R, tag="w_i")
            wv_n = ws.tile([128, 256], F32R, tag="w_n")
            nc.sync.dma_start(wv_r[:], wv_d[0, 128 * cj:128 * (cj + 1), :])
            nc.sync.dma_start(wv_i[:], wv_d[1, 128 * cj:128 * (cj + 1), :])
            nc.sync.dma_start(wv_n[:], wv_d[2, 128 * cj:128 * (cj + 1), :])
            st = cj == 0
            for kc in kcg:
                k0, szk = K_CHUNKS[kc]
                pr, pi = pps[kc]
                nc.tensor.matmul(pr[:szk, :256], _r(xnTr[:, cj, k0:k0 + szk]),
                                 _r(wv_r[:]), start=st, stop=False)
                nc.tensor.matmul(pr[:szk, :256], _r(xnTi[:, cj, k0:k0 + szk]),
                                 _r(wv_n[:]), start=False, stop=False)
                nc.tensor.matmul(pi[:szk, :256], _r(xnTr[:, cj, k0:k0 + szk]),
                                 _r(wv_i[:]), start=st, stop=False)
                nc.tensor.matmul(pi[:szk, :256], _r(xnTi[:, cj, k0:k0 + szk]),
                                 _r(wv_r[:]), start=False, stop=False)
        for kc in kcg:
            k0, szk = K_CHUNKS[kc]
            pr, pi = pps[kc]
            nc.tensor.matmul(pr[:szk, :256], _r(ones_row[:, :szk]), _r(bv_re[:]),
                             start=False, stop=True)
            nc.tensor.matmul(pi[:szk, :256], _r(ones_row[:, :szk]), _r(bv_im[:]),
                             start=False, stop=True)
            vr_v = pr[:szk, :256].rearrange("p (h d) -> p h d", h=4)
            vi_v = pi[:szk, :256].rearrange("p (h d) -> p h d", h=4)
            nc.scalar.copy(vpk[:szk, kc, :, 0:64], vr_v)
            nc.scalar.copy(vpk[:szk, kc, :, 64:128], vi_v)

    for s_ in range(3):
        nc.gpsimd.dma_start(wk_sb[:, :, s_, :],
                            wk_d[s_].rearrange("(j p) n -> p j n", p=128))
    for mc in range(5):
        t0, sz = MC[mc]
        _ln_sums(nc, work, st5, acc_re[:, mc], acc_im[:, mc], sz, mc)
    emit_qproj(*Q_CHUNKS[0])
    emit_qproj(*Q_CHUNKS[1])
    lnst[0] = _ln_stats(nc, stats, st5)
    for mc in range(5):
        emit_ln(mc)
        if mc > 1:
            emit_qproj(*Q_CHUNKS[mc])
        if mc == 2:
            emit_kproj(0)
        elif mc == 3:
            emit_vproj((0, 1, 2))
        elif mc == 4:
            emit_kproj(1)
            emit_vproj((3, 4))

    # phase-1/2 scratch pools release their SBUF before the attention
    # pools are opened (the tile framework time-shares the space)
    p1ctx.close()
    sm = ctx.enter_context(tc.tile_pool(name="sm", bufs=2))

    # =====================================================================
    # Phase 3: attention (S^T layout, f32r scores), softmax over |.|,
    # fused partial output projection per q-chunk.
    # =====================================================================
    wp_sb = big.tile([128, 3, 2, C], BF16, tag="wp")
    nc.sync.dma_start(wp_sb[:], wp_d.rearrange("s (j p) n -> p s j n", p=128))

    def emit_front_mm(q0, nq, hp):
        sbufA = sm.tile([128, 3, 2, 512], F16, tag="sbufA", bufs=2)
        sbufB = sm.tile([128, 2, 2, 512], F16, tag="sbufB", bufs=2)
        for kc in range(5):
            k0, szk = K_CHUNKS[kc]
            sbuf, kco = (sbufA, kc) if kc < 3 else (sbufB, kc - 3)
            for i in range(2):
                rs = slice(64 * i, 64 * (i + 1))
                sre = psum.tile([128, 512], F32, tag="bank")
                sim = psum.tile([128, 512], F32, tag="bank")
                nc.tensor.matmul(sre[:szk, :nq], _r(kTr[rs, hp, k0:k0 + szk]),
                                 qT_sb[rs, 0, hp, q0:q0 + nq], start=True,
                                 stop=False)
                nc.tensor.matmul(sim[:szk, :nq], _r(kTi[rs, hp, k0:k0 + szk]),
                                 qT_sb[rs, 0, hp, q0:q0 + nq], start=True,
                                 stop=False)
                nc.tensor.matmul(sre[:szk, :nq], _r(kTin[rs, hp, k0:k0 + szk]),
                                 qT_sb[rs, 1, hp, q0:q0 + nq], start=False,
                                 stop=True)
                nc.tensor.matmul(sim[:szk, :nq], _r(kTr[rs, hp, k0:k0 + szk]),
                                 qT_sb[rs, 1, hp, q0:q0 + nq], start=False,
                                 stop=True)
                t1 = sm.tile([128, 512], F16, tag="t1", bufs=2)
                t2 = sm.tile([128, 512], F16, tag="t2", bufs=2)
                nc.vector.tensor_copy(t1[:szk, :nq], sre[:szk, :nq])
                nc.vector.tensor_copy(t2[:szk, :nq], sim[:szk, :nq])
                u1 = sm.tile([128, 512], F16, tag="u1", bufs=1)
                u2 = sm.tile([128, 512], F16, tag="u2", bufs=1)
                nc.gpsimd.tensor_mul(u1[:szk, :nq], t1[:szk, :nq],
                                     t1[:szk, :nq])
                nc.gpsimd.tensor_mul(u2[:szk, :nq], t2[:szk, :nq],
                                     t2[:szk, :nq])
                nc.gpsimd.tensor_add(sbuf[:szk, kco, i, :nq], u1[:szk, :nq],
                                     u2[:szk, :nq])
        return sbufA, sbufB

    def emit_chain(q0, nq, hp, sbufA, sbufB):
        # |z| = exp(0.5 ln(|z|^2)); Ln/Exp/Square/Copy all live in the
        # natural_log_exp_and_others table, so no table reloads here.
        # constant shift keeps exp sums in f32 range; softmax is
        # shift-invariant so the result is exact
        ebuf = sm.tile([128, 5, 2, 512], BF16, tag="ebuf", bufs=4)
        for (sb, kl, kn) in ((sbufA, 0, 3), (sbufB, 3, 2)):
            ks = slice(kl, kl + kn)
            ub = sm.tile([128, 3, 2, 512], F32, tag="ubuf", bufs=1)
            un = ub[:, :kn, :, :nq]
            nc.scalar.activation(un, sb[:, :kn, :, :nq], AF.Ln)
            nc.scalar.activation(un, un, AF.Exp, scale=0.5)
            nc.scalar.activation(ebuf[:, ks, :, :nq], un, AF.Exp,
                                 bias=nbias[:])
        return ebuf

    def emit_backmm(q0, nq, hp, ebuf):
        op0 = psum.tile([128, 512], F32, tag="bank", name="op0")
        op1 = psum.tile([128, 512], F32, tag="bank", name="op1")
        dn = psum.tile([128, 512], F32, tag="bank", name="dn")
        for kc in range(5):
            k0, szk = K_CHUNKS[kc]
            for i in range(2):
                hh = 2 * hp + i
                opt = op0 if i == 0 else op1
                nc.tensor.matmul(opt[:, :nq], vpk[:szk, kc, hh, :],
                                 ebuf[:szk, kc, i, :nq], start=kc == 0,
                                 stop=kc == 4)
                nc.tensor.matmul(dn[32 * i:32 * i + 1, :nq], ones_col[:szk, :],
                                 ebuf[:szk, kc, i, :nq], start=kc == 0,
                                 stop=kc == 4, tile_position=(0, 32 * i))
        # tail stage 1: log-denominators on ACT (queued right after the
        # previous chain) + attn-out drained from PSUM by ACT.  One [33,*]
        # op covers both heads' rows (0 and 32); rows in between are
        # unread garbage.
        rh = stats.tile([33, 512], F32R, tag="lnd")
        nc.scalar.activation(rh[:, :nq], dn[0:33, :nq], AF.Ln)
        osbs = []
        for i, opt in enumerate((op0, op1)):
            osb = sm.tile([128, 512], F32, tag="osb", bufs=3)
            nc.scalar.copy(osb[:, :nq], opt[:, :nq])
            osbs.append(osb)
        return rh, osbs

    def emit_backtail(q0, nq, hp, rh, osbs):
        # tail stage 2 (one iteration later, so the PE never waits on ACT):
        # 1/dn broadcast + normalize
        otr = sm.tile([128, 512], BF16, tag="otr", bufs=4)
        oti = sm.tile([128, 512], BF16, tag="oti", bufs=4)
        nc.scalar.activation(rh[:, :nq], rh[:, :nq], AF.Exp, scale=-1.0)
        for i in range(2):
            rbp = psum.tile([128, 512], F32, tag="bank")
            nc.tensor.matmul(rbp[:, :nq], _r(ones2[32 * i:32 * i + 1, :]),
                             _r(rh[32 * i:32 * i + 1, :nq]),
                             start=True, stop=True)
            rb = sm.tile([128, 512], F32, tag="rb", bufs=1)
            nc.vector.tensor_copy(rb[:, :nq], rbp[:, :nq])
            osb = osbs[i]
            rs = slice(64 * i, 64 * (i + 1))
            nc.gpsimd.tensor_mul(otr[rs, :nq], osb[0:64, :nq], rb[0:64, :nq])
            nc.gpsimd.tensor_mul(oti[rs, :nq], osb[64:128, :nq], rb[64:128, :nq])
        return otr, oti

    def emit_proj(q0, nq, outs):
        for cc in range(4):
            cs = slice(128 * cc, 128 * (cc + 1))
            pr = psum.tile([128, 512], F32, tag="bank")
            pi = psum.tile([128, 512], F32, tag="bank")
            for hp in range(2):
                otr, oti = outs[hp]
                st = hp == 0
                sp = hp == 1
                nc.tensor.matmul(pr[:, :nq], wp_sb[:, 0, hp, cs],
                                 otr[:, :nq], start=st, stop=False)
                nc.tensor.matmul(pr[:, :nq], wp_sb[:, 2, hp, cs],
                                 oti[:, :nq], start=False, stop=sp)
                nc.tensor.matmul(pi[:, :nq], wp_sb[:, 1, hp, cs],
                                 otr[:, :nq], start=st, stop=False)
                nc.tensor.matmul(pi[:, :nq], wp_sb[:, 0, hp, cs],
                                 oti[:, :nq], start=False, stop=sp)
            o1 = sm.tile([128, 512], F16, tag="cp_r16")
            o2 = sm.tile([128, 512], F16, tag="cp_i16")
            nc.vector.tensor_copy(o1[:, :nq], pr[:, :nq])
            nc.vector.tensor_copy(o2[:, :nq], pi[:, :nq])
            nc.gpsimd.dma_start(outT_d[0, cs, q0:q0 + nq], o1[:, :nq])
            nc.sync.dma_start(outT_d[1, cs, q0:q0 + nq], o2[:, :nq])

    # software pipeline, 2 iterations deep, with a two-stage back tail:
    # iteration j runs the score matmuls of j, the attn@v of j-2, the
    # denominator stage-1 (ACT) of j-2, the normalize stage-2 of j-3 and
    # the output projection of ~j-4, so no engine ever waits on another
    # iteration's serial chain.
    fronts = []
    tails = []
    pending = {}
    ready = []
    def flush_back():
        q0_, nq_, hp_, ebA = fronts.pop(0)
        rh_, osbs_ = emit_backmm(q0_, nq_, hp_, ebA)
        tails.append((q0_, nq_, hp_, rh_, osbs_))
    def flush_tail():
        q0_, nq_, hp_, rhs_t, osbs = tails.pop(0)
        pending.setdefault(q0_, [None, None])[hp_] = \
            emit_backtail(q0_, nq_, hp_, rhs_t, osbs)
        if hp_ == 1:
            ready.append((q0_, nq_))
    def flush_proj():
        q_, n_ = ready.pop(0)
        emit_proj(q_, n_, pending.pop(q_))
    for (q0, nq) in Q_CHUNKS:
        for hp in range(2):
            sbufA, sbufB = emit_front_mm(q0, nq, hp)
            if len(fronts) >= 3:
                flush_back()
            if len(tails) >= 2:
                flush_tail()
            if len(ready) >= 2:
                flush_proj()
            fronts.append((q0, nq, hp, emit_chain(q0, nq, hp, sbufA, sbufB)))
    while fronts:
        flush_back()
        while len(tails) >= 2:
            flush_tail()
    while tails:
        flush_tail()
    while ready:
        flush_proj()

    ctx.close()


# =========================================================================
# Host side
# =========================================================================

def _f32(x):
    return np.ascontiguousarray(x, dtype=np.float32)


def _bf(x):
    return np.asarray(x, dtype=ml_dtypes.bfloat16)


def host_prep(x_re, x_im, Wq, Wkv, Wproj, bproj, sr_w, sr_b, gain, bias):
    x_re = np.asarray(x_re)
    x_im = np.asarray(x_im)
    Wq = np.asarray(Wq)
    Wkv = np.asarray(Wkv)
    Wproj = np.asarray(Wproj)
    sr_w = np.asarray(sr_w)
    sr_b = np.asarray(sr_b)
    gain = np.asarray(gain)
    bias = np.asarray(bias)

    Wkv_eff = gain[:, None] * Wkv
    bkv_full = bias @ Wkv
    Wc = sr_w.transpose(2, 3, 1, 0).reshape(4 * C, C)

    def planes3f(w):
        return np.stack([_f32(w.real), _f32(w.imag), _f32(-w.imag)])

    def planes3b(w):
        return np.stack([_bf(w.real), _bf(w.imag), _bf(-w.imag)])

    in_maps = []
    for core in range(8):
        b, g = core // 2, core % 2
        cols = slice(256 * g, 256 * (g + 1))
        wk_c = Wkv_eff[:, :C][:, cols] * SCALE
        wv_c = Wkv_eff[:, C:][:, cols]
        bk_c = bkv_full[:C][cols] * SCALE
        bv_c = bkv_full[C:][cols]
        xs_c = np.stack([x_re[b].T, x_im[b].T])  # [2, C, N]
        xsp = xs_c.reshape(2, C, HR, 2, HR, 2)
        xP = np.stack([xsp[:, :, :, p, :, q].reshape(2, C, NK)
                       for p in range(2) for q in range(2)], axis=1)
        m = {
            "xT": _f32(xs_c),
            "xP": _f32(xP.reshape(2, 4 * C, NK)),
            "wc": planes3f(Wc),
            "srb": np.stack([_f32(sr_b.real), _f32(sr_b.imag)]),
            "ones": np.ones((1, 512), np.float32),
            "wq": planes3f(Wq[:, cols]),
            "wk": planes3f(wk_c),
            "wv": planes3f(wv_c),
            "wp": planes3b(Wproj[256 * g:256 * (g + 1), :]),
            "bkv": np.stack([
                np.stack([_f32(bk_c.real), _f32(bv_c.real)]),
                np.stack([_f32(bk_c.imag), _f32(bv_c.imag)]),
            ]),
        }
        in_maps.append(m)
    return in_maps


_NC_CACHE = None


def _get_nc():
    global _NC_CACHE
    if _NC_CACHE is None:
        _NC_CACHE = build_nc()
    return _NC_CACHE


def kernel(x_re, x_im, Wq, Wkv, Wproj, bproj, sr_w, sr_b, gain, bias, H, W):
    from concourse.bass_utils import run_bass_kernel_spmd

    nc = _get_nc()
    in_maps = host_prep(x_re, x_im, Wq, Wkv, Wproj, bproj, sr_w, sr_b, gain, bias)
    res = run_bass_kernel_spmd(nc, in_maps, list(range(8)))
    bproj = np.asarray(bproj)
    out = np.zeros((B, N, C), dtype=np.complex64)
    for b in range(B):
        p0 = res.results[2 * b]["outT"].astype(np.float32)
        p1 = res.results[2 * b + 1]["outT"].astype(np.float32)
        acc = (p0[0] + p1[0]).T + 1j * (p0[1] + p1[1]).T
        out[b] = acc + bproj[None, :]
    return out



# revision 65
# speedup vs baseline: 1.0084x; 1.0084x over previous
"""Trainium2 Bass kernel for complex-valued spatial-reduction attention.

x: [B=4, N=2304, C=512] complex64 (re/im f32 planes), H=W=48, 8 heads,
head_dim 64, sr_ratio 2 -> Nk=576.

Sharding: 8 cores = 4 batches x 2 head-groups (4 heads each). Each core:
sr-conv over full C, complex LayerNorm, q/k/v for its heads,
softmax(|q.k^T|) attention, attn @ v, partial output projection.
Host sums the two partials per batch and adds bproj.

Precision: softmax logits reach |a| ~ 37, so the conv -> LN-stats ->
q/kv -> scores chain runs in f32r matmuls (f32-equivalent precision at
bf16-class speed for free dims >= 256, measured on HW). E/v/attn@v/proj
use bf16.

Structure (one pass, phases overlap via per-engine emission order):
- conv decomposed by patch position, accumulated in SBUF so the 12.6MB
  weight streams exactly once; LN stats batched across all 5 row chunks
- q^T stays resident in SBUF (no DRAM round-trip); wq/wk cached
- attention runs a 3-iteration-deep software pipeline with a two-stage
  normalization tail and the output projection folded in, so the softmax
  chain's latency never blocks the PE
- activation-table trick: phase 1 runs entirely out of sqrt_and_others,
  the attention phase out of natural_log_exp_and_others (2 table loads
  total instead of 94)
"""

import contextlib

import numpy as np
import ml_dtypes

import concourse.bass as bass
import concourse.mybir as mybir
import concourse.tile as tile
from concourse import bacc
from concourse.masks import make_identity

BF16 = mybir.dt.bfloat16
F16 = mybir.dt.float16
F32 = mybir.dt.float32
F32R = mybir.dt.float32r
AF = mybir.ActivationFunctionType
ALU = mybir.AluOpType

B, N, C, HEADS, HD, SR = 4, 2304, 512, 8, 64, 2
NK = 576
HR = 24
EPS = 1e-5
SCALE = HD ** -0.5  # folded into Wk host-side

K_CHUNKS = [(0, 128), (128, 128), (256, 128), (384, 128), (512, 64)]
Q_CHUNKS = [(0, 512), (512, 512), (1024, 512), (1536, 512), (2048, 256)]


def _r(ap):
    return ap.bitcast(F32R)


def _trim_act_tables(arch):
    """Steer the act-table placement pass to sets that cover whole phases.

    The greedy inserter picks the first table containing the missing
    function: Ln -> natural_log (5), Exp -> exp_and_others (0), so the
    attention loop ping-pongs 0<->5 (~94 loads, ~1.3us each).  Trimming
    the cached table metadata makes the first match for Ln AND Exp be
    natural_log_exp_and_others (6), and Sign resolve to sqrt_and_others
    (3) alongside phase 1's Sqrt.  The real HW tables are unchanged and
    genuinely contain every function executed under them (6 = union of
    0 and 5), so the compiled program remains valid -- it just loads 2
    tables instead of 94.
    """
    from concourse.hw_specs import get_activation_tables

    tabs = get_activation_tables(arch)
    names = list(tabs)
    tabs[names[0]].discard(AF.Sign)
    tabs[names[1]].discard(AF.Sign)
    tabs[names[2]].discard(AF.Sign)
    tabs[names[5]].discard(AF.Ln)
    tabs[names[0]].discard(AF.Exp)


def build_nc():
    nc = bacc.Bacc("TRN2", target_bir_lowering=False, debug=False, num_devices=8)
    _trim_act_tables(nc.m.arch)

    xT_d = nc.dram_tensor("xT", [2, C, N], F32R, kind="ExternalInput")
    xP_d = nc.dram_tensor("xP", [2, 4 * C, NK], F32R, kind="ExternalInput")
    wc_d = nc.dram_tensor("wc", [3, 4 * C, C], F32R, kind="ExternalInput")
    srb_d = nc.dram_tensor("srb", [2, C], F32R, kind="ExternalInput")
    ones_d = nc.dram_tensor("ones", [1, 512], F32R, kind="ExternalInput")
    wq_d = nc.dram_tensor("wq", [3, C, 256], F32R, kind="ExternalInput")
    wk_d = nc.dram_tensor("wk", [3, C, 256], F32R, kind="ExternalInput")
    wv_d = nc.dram_tensor("wv", [3, C, 256], F32R, kind="ExternalInput")
    wp_d = nc.dram_tensor("wp", [3, 256, C], BF16, kind="ExternalInput")
    bkv_d = nc.dram_tensor("bkv", [2, 2, 256], F32R, kind="ExternalInput")
    outT_d = nc.dram_tensor("outT", [2, C, N], F16, kind="ExternalOutput")

    with tile.TileContext(nc) as tc:
        _body(nc, tc, xT_d, xP_d, wc_d, srb_d, ones_d, wq_d, wk_d, wv_d, wp_d,
              bkv_d, outT_d)

    nc.compile()
    return nc


def _ln_sums(nc, work, st5, re_sb, im_sb, sz, mc):
    """Row sums/moments for one [sz, C] chunk -> column mc of the [128,5]
    stat matrices."""
    sum_r5, sum_i5, sxx5, sii5, sxi5 = st5
    nc.vector.tensor_reduce(sum_r5[:sz, mc:mc + 1], re_sb[:sz],
                            mybir.AxisListType.X, ALU.add)
    nc.vector.tensor_reduce(sum_i5[:sz, mc:mc + 1], im_sb[:sz],
                            mybir.AxisListType.X, ALU.add)
    junk = work.tile([128, C], F32, tag="ln_junk", bufs=1)
    nc.gpsimd.tensor_mul(junk[:sz], re_sb[:sz], re_sb[:sz])
    nc.vector.tensor_reduce(sxx5[:sz, mc:mc + 1], junk[:sz],
                            mybir.AxisListType.X, ALU.add)
    junk2 = work.tile([128, C], F32, tag="ln_junk2", bufs=1)
    nc.gpsimd.tensor_mul(junk2[:sz], im_sb[:sz], im_sb[:sz])
    nc.vector.tensor_reduce(sii5[:sz, mc:mc + 1], junk2[:sz],
                            mybir.AxisListType.X, ALU.add)
    junk3 = work.tile([128, C], F32, tag="ln_junk3", bufs=1)
    nc.gpsimd.tensor_mul(junk3[:sz], re_sb[:sz], im_sb[:sz])
    nc.vector.tensor_reduce(sxi5[:sz, mc:mc + 1], junk3[:sz],
                            mybir.AxisListType.X, ALU.add)


def _ln_stats(nc, stats, st5):
    """Complex-LN statistics for all 5 chunks at once on [128, 5] tiles.
    Unwritten rows of the short chunk flow garbage through; their columns
    are never read back."""
    sum_r, sum_i, sxx, sii, sxi = st5
    inv_c = 1.0 / C
    W = 5

    def T(tag):
        return stats.tile([128, W], F32, tag=tag, name=tag)

    mr, mi = T("mr"), T("mi")
    nc.gpsimd.tensor_scalar_mul(mr[:], sum_r[:], inv_c)
    nc.gpsimd.tensor_scalar_mul(mi[:], sum_i[:], inv_c)
    vre, vim, tA, tB = T("vre"), T("vim"), T("tA"), T("tB")
    nc.gpsimd.tensor_sub(tA[:], sxx[:], sii[:])
    nc.gpsimd.tensor_scalar_mul(tA[:], tA[:], inv_c)
    nc.gpsimd.tensor_mul(vre[:], mr[:], mr[:])
    nc.gpsimd.tensor_mul(tB[:], mi[:], mi[:])
    nc.gpsimd.tensor_sub(vre[:], vre[:], tB[:])
    nc.gpsimd.tensor_sub(vre[:], tA[:], vre[:])
    nc.gpsimd.tensor_scalar_add(vre[:], vre[:], EPS)
    nc.gpsimd.tensor_mul(tB[:], mr[:], mi[:])
    nc.gpsimd.tensor_scalar_mul(tB[:], tB[:], 2.0)
    nc.gpsimd.tensor_scalar_mul(vim[:], sxi[:], 2.0 * inv_c)
    nc.gpsimd.tensor_sub(vim[:], vim[:], tB[:])
    r2 = T("r2")
    nc.gpsimd.tensor_mul(r2[:], vre[:], vre[:])
    nc.gpsimd.tensor_mul(tB[:], vim[:], vim[:])
    nc.gpsimd.tensor_add(r2[:], r2[:], tB[:])

    def _sqrt_newton(out, x, sc):
        # y0 = LUT sqrt(sc*x); y1 = 0.5*(y0 + sc*x/y0)  (one Newton step)
        y0 = T("nw_y0")
        nc.scalar.activation(y0[:], x[:], AF.Sqrt, scale=sc)
        yr = T("nw_yr")
        nc.vector.tensor_scalar_add(y0[:], y0[:], 1e-30)
        nc.vector.reciprocal(yr[:], y0[:])
        nc.vector.tensor_mul(yr[:], yr[:], x[:])
        if sc != 1.0:
            nc.vector.tensor_scalar_mul(yr[:], yr[:], sc)
        nc.vector.tensor_add(out[:], y0[:], yr[:])
        nc.vector.tensor_scalar_mul(out[:], out[:], 0.5)

    rr = T("rr")
    _sqrt_newton(rr, r2, 1.0)
    srt, sia = T("srt"), T("sia")
    nc.vector.tensor_add(tA[:], rr[:], vre[:])
    _sqrt_newton(srt, tA, 0.5)
    nc.vector.tensor_sub(tA[:], rr[:], vre[:])
    _sqrt_newton(sia, tA, 0.5)
    sgn = T("sgn")
    nc.scalar.activation(sgn[:], vim[:], AF.Sign)
    nc.vector.tensor_mul(sia[:], sia[:], sgn[:])
    rin = T("rin")
    nc.vector.reciprocal(rin[:], rr[:])
    wr, wn = T("wr"), T("wn")  # wn = -w_im
    nc.vector.tensor_mul(wr[:], srt[:], rin[:])
    nc.vector.tensor_mul(wn[:], sia[:], rin[:])
    return mr, mi, wr, wn


def _ln_norm(nc, work, lnst, re_sb, im_sb, sz, mc):
    """Apply the complex LN transform for chunk mc -> (xnr, xni)."""
    mr5, mi5, wr5, wn5 = lnst
    mr = mr5[:, mc:mc + 1]
    mi = mi5[:, mc:mc + 1]
    wr = wr5[:, mc:mc + 1]
    wn = wn5[:, mc:mc + 1]
    aT = work.tile([128, C], F32, tag="ln_junk", bufs=1)
    bT = work.tile([128, C], F32, tag="ln_junk2", bufs=1)
    xnr = work.tile([128, C], F32, tag="ln_xnr", bufs=1)
    xni = work.tile([128, C], F32, tag="ln_xni", bufs=1)
    nc.gpsimd.tensor_scalar(aT[:sz], re_sb[:sz], mr[:sz], wr[:sz],
                            ALU.subtract, ALU.mult)
    nc.gpsimd.tensor_scalar(bT[:sz], im_sb[:sz], mi[:sz], wn[:sz],
                            ALU.subtract, ALU.mult)
    nc.gpsimd.tensor_add(xnr[:sz], aT[:sz], bT[:sz])
    nc.gpsimd.tensor_scalar(aT[:sz], re_sb[:sz], mr[:sz], wn[:sz],
                            ALU.subtract, ALU.mult)
    nc.gpsimd.tensor_scalar(bT[:sz], im_sb[:sz], mi[:sz], wr[:sz],
                            ALU.subtract, ALU.mult)
    nc.gpsimd.tensor_sub(xni[:sz], bT[:sz], aT[:sz])
    return xnr, xni


def _body(nc, tc, xT_d, xP_d, wc_d, srb_d, ones_d, wq_d, wk_d, wv_d, wp_d,
          bkv_d, outT_d):
    ctx = contextlib.ExitStack()
    consts = ctx.enter_context(tc.tile_pool(name="consts", bufs=1))
    big = ctx.enter_context(tc.tile_pool(name="big", bufs=1))
    work = ctx.enter_context(tc.tile_pool(name="work", bufs=2))
    stats = ctx.enter_context(tc.tile_pool(name="stats", bufs=2))
    psum = ctx.enter_context(tc.tile_pool(name="psum", bufs=8, space="PSUM"))
    # phase-1/2-only pools -- closed before the attention pools open so the
    # SBUF is time-shared
    p1ctx = contextlib.ExitStack()
    cv = p1ctx.enter_context(tc.tile_pool(name="cv", bufs=1))
    qs = p1ctx.enter_context(tc.tile_pool(name="qs", bufs=2))
    ws = p1ctx.enter_context(tc.tile_pool(name="ws", bufs=2))

    # ---- constants ----
    ident = consts.tile([128, 128], F32, tag="ident")
    make_identity(nc, ident)
    ones_col = consts.tile([128, 1], BF16, tag="ones_col")
    nc.vector.memset(ones_col, 1.0)
    ones_row = consts.tile([1, 512], F32R, tag="ones_row")
    nbias = consts.tile([128, 1], F32, tag="nbias")
    nc.vector.memset(nbias, -50.0)
    ones2 = consts.tile([33, 128], F32, tag="ones2")
    nc.vector.memset(ones2, 1.0)

    srb_re = cv.tile([1, C], F32R, tag="srb_re")
    srb_im = cv.tile([1, C], F32R, tag="srb_im")
    bk_re = cv.tile([1, 256], F32R, tag="bk_re")
    bk_im = cv.tile([1, 256], F32R, tag="bk_im")
    bv_re = cv.tile([1, 256], F32R, tag="bv_re")
    bv_im = cv.tile([1, 256], F32R, tag="bv_im")

    # ---- persistent SBUF ----
    xnTr = big.tile([128, 4, NK], F32R, tag="xnTr")
    xnTi = big.tile([128, 4, NK], F32R, tag="xnTi")
    kTr = big.tile([128, 2, NK], F32R, tag="kTr")
    kTi = big.tile([128, 2, NK], F32R, tag="kTi")
    kTin = big.tile([128, 2, NK], F32R, tag="kTin")
    vpk = big.tile([128, 5, 4, 128], BF16, tag="vpk")
    # q^T lives in SBUF for the whole kernel: [plane, half, n]
    qT_sb = big.tile([128, 2, 2, N], F32R, tag="qT_sb")
    # wq cached across the 5 q-chunks: [cj, plane, out]
    wq_sb = big.tile([128, 4, 3, 256], F32R, tag="wq_sb")

    xT_v = [xT_d[pl].rearrange("(j p) t -> p j t", p=128) for pl in (0, 1)]

    # =====================================================================
    # Phase 1: sr-conv, decomposed by patch position so wc streams ONCE.
    # Accumulate the 4 positions in SBUF (acc_re/acc_im), LayerNorm reads
    # the accumulators directly.  LN chunks are interleaved with q-proj
    # chunks so the PE has matmul work while LN runs on DVE/ACT.
    # =====================================================================
    MC = [(0, 120), (120, 120), (240, 120), (360, 120), (480, 96)]
    acc_re = cv.tile([128, 5, C], F32, tag="acc_re")
    acc_im = cv.tile([128, 5, C], F32, tag="acc_im")

    for pos in range(4):
        if pos == 1:
            # deferred constant loads: nothing upstream needs them until
            # the pos-3 bias matmuls / kproj, so the pos-0 conv tiles win
            # the DMA queues
            nc.sync.dma_start(ones_row[:], ones_d[:])
            nc.sync.dma_start(srb_re[:], srb_d[0:1, :])
            nc.sync.dma_start(srb_im[:], srb_d[1:2, :])
            nc.sync.dma_start(bk_re[:], bkv_d[0:1, 0, :])
            nc.sync.dma_start(bv_re[:], bkv_d[0:1, 1, :])
            nc.sync.dma_start(bk_im[:], bkv_d[1:2, 0, :])
            nc.sync.dma_start(bv_im[:], bkv_d[1:2, 1, :])
        wts = []
        xps = []
        for j in range(4):
            r0 = 512 * pos + 128 * j
            w = cv.tile([128, 3, C], F32R, tag="wpos", bufs=4)
            nc.gpsimd.dma_start(w[:, 0], wc_d[0, r0:r0 + 128, :])
            nc.sync.dma_start(w[:, 1], wc_d[1, r0:r0 + 128, :])
            nc.gpsimd.dma_start(w[:, 2], wc_d[2, r0:r0 + 128, :])
            xp = cv.tile([128, 2, NK], F32R, tag="xpos", bufs=4)
            nc.sync.dma_start(xp[:, 0], xP_d[0, r0:r0 + 128, :])
            nc.gpsimd.dma_start(xp[:, 1], xP_d[1, r0:r0 + 128, :])
            wts.append(w)
            xps.append(xp)
        for (t0, sz) in MC:
            cre = psum.tile([128, C], F32, tag="bank")
            cim = psum.tile([128, C], F32, tag="bank")
            for j in range(4):
                w, xp = wts[j], xps[j]
                pat_r = xp[:, 0, t0:t0 + sz]
                pat_i = xp[:, 1, t0:t0 + sz]
                st = j == 0
                nc.tensor.matmul(cre[:sz, :], pat_r, w[:, 0], start=st,
                                 stop=False)
                nc.tensor.matmul(cim[:sz, :], pat_r, w[:, 1], start=st,
                                 stop=False)
                nc.tensor.matmul(cre[:sz, :], pat_i, w[:, 2], start=False,
                                 stop=False)
                nc.tensor.matmul(cim[:sz, :], pat_i, w[:, 0], start=False,
                                 stop=j == 3 and pos != 3)
            if pos == 3:
                nc.tensor.matmul(cre[:sz, :], _r(ones_row[:, :sz]),
                                 _r(srb_re[:]), start=False, stop=True)
                nc.tensor.matmul(cim[:sz, :], _r(ones_row[:, :sz]),
                                 _r(srb_im[:]), start=False, stop=True)
            mc = MC.index((t0, sz))
            if pos == 0:
                nc.scalar.copy(acc_re[:sz, mc], cre[:sz, :])
                nc.scalar.copy(acc_im[:sz, mc], cim[:sz, :])
            else:
                nc.vector.tensor_add(acc_re[:sz, mc], cre[:sz, :],
                                     acc_re[:sz, mc])
                nc.vector.tensor_add(acc_im[:sz, mc], cim[:sz, :],
                                     acc_im[:sz, mc])

    for s_ in range(3):
        nc.sync.dma_start(wq_sb[:, :, s_, :],
                          wq_d[s_].rearrange("(j p) n -> p j n", p=128))

    st5 = tuple(stats.tile([128, 5], F32, tag=t_, bufs=1, name=t_)
                for t_ in ("sum_r5", "sum_i5", "sxx5", "sii5", "sxi5"))
    lnst = [None]

    def emit_ln(mc):
        t0, sz = MC[mc]
        xnr, xni = _ln_norm(nc, work, lnst[0], acc_re[:, mc],
                            acc_im[:, mc], sz, mc)
        for cj in range(4):
            for srcT, dst in ((xnr, xnTr), (xni, xnTi)):
                pt = psum.tile([128, 128], F32, tag="bank")
                nc.tensor.transpose(pt[:, :sz],
                                    srcT[:sz, 128 * cj:128 * (cj + 1)],
                                    ident[:sz, :sz])
                nc.scalar.copy(dst[:, cj, t0:t0 + sz], pt[:, :sz])

    # =====================================================================
    # Phase 1b: q-projection (f32r), x^T streamed, q^T kept in SBUF
    # =====================================================================
    def emit_qproj(q0, nq):
        prs = []
        for half in range(2):
            prs.append((psum.tile([128, 512], F32, tag="bank", name=f"qpr{half}"),
                        psum.tile([128, 512], F32, tag="bank", name=f"qpi{half}")))
        for cj in range(4):
            xq_r = qs.tile([128, 512], F32R, tag="xq_r", bufs=2)
            xq_i = qs.tile([128, 512], F32R, tag="xq_i", bufs=2)
            nc.gpsimd.dma_start(xq_r[:, :nq], xT_v[0][:, cj, q0:q0 + nq])
            nc.sync.dma_start(xq_i[:, :nq], xT_v[1][:, cj, q0:q0 + nq])
            st = cj == 0
            sp = cj == 3
            for half in range(2):
                hs = slice(128 * half, 128 * (half + 1))
                pr, pi = prs[half]
                nc.tensor.matmul(pr[:, :nq], wq_sb[:, cj, 0, hs], _r(xq_r[:, :nq]),
                                 start=st, stop=False)
                nc.tensor.matmul(pr[:, :nq], wq_sb[:, cj, 2, hs], _r(xq_i[:, :nq]),
                                 start=False, stop=sp)
                nc.tensor.matmul(pi[:, :nq], wq_sb[:, cj, 1, hs], _r(xq_r[:, :nq]),
                                 start=st, stop=False)
                nc.tensor.matmul(pi[:, :nq], wq_sb[:, cj, 0, hs], _r(xq_i[:, :nq]),
                                 start=False, stop=sp)
        for half in range(2):
            pr, pi = prs[half]
            nc.scalar.copy(qT_sb[:, 0, half, q0:q0 + nq], pr[:, :nq])
            nc.scalar.copy(qT_sb[:, 1, half, q0:q0 + nq], pi[:, :nq])

    # =====================================================================
    # Phase 2: k^T (two NK-halves of 288, so every matmul free dim >= 256)
    # and v projections (f32r).  Emitted interleaved with LN chunks.
    # =====================================================================
    wk_sb = cv.tile([128, 4, 3, 256], F32R, tag="wk_sb")

    def emit_kproj(nkh):
        n0, nn = 288 * nkh, 288
        for half in range(2):
            hs = slice(128 * half, 128 * (half + 1))
            pr = psum.tile([128, 512], F32, tag="bank")
            pi = psum.tile([128, 512], F32, tag="bank")
            for cj in range(4):
                st = cj == 0
                nc.tensor.matmul(pr[:, :nn], wk_sb[:, cj, 0, hs],
                                 _r(xnTr[:, cj, n0:n0 + nn]), start=st,
                                 stop=False)
                nc.tensor.matmul(pr[:, :nn], wk_sb[:, cj, 2, hs],
                                 _r(xnTi[:, cj, n0:n0 + nn]), start=False,
                                 stop=False)
                nc.tensor.matmul(pi[:, :nn], wk_sb[:, cj, 1, hs],
                                 _r(xnTr[:, cj, n0:n0 + nn]), start=st,
                                 stop=False)
                nc.tensor.matmul(pi[:, :nn], wk_sb[:, cj, 0, hs],
                                 _r(xnTi[:, cj, n0:n0 + nn]), start=False,
                                 stop=False)
            nc.tensor.matmul(pr[:, :nn], _r(bk_re[:, hs]), _r(ones_row[:, :nn]),
                             start=False, stop=True)
            nc.tensor.matmul(pi[:, :nn], _r(bk_im[:, hs]), _r(ones_row[:, :nn]),
                             start=False, stop=True)
            nc.scalar.copy(kTr[:, half, n0:n0 + nn], pr[:, :nn])
            nc.scalar.copy(kTi[:, half, n0:n0 + nn], pi[:, :nn])
            nc.scalar.mul(kTin[:, half, n0:n0 + nn], pi[:, :nn], -1.0)

    def emit_vproj(kcg):
        pps = {}
        for kc in kcg:
            pps[kc] = (psum.tile([128, 512], F32, tag="bank", name=f"vpr{kc}"),
                       psum.tile([128, 512], F32, tag="bank", name=f"vpi{kc}"))
        for cj in range(4):
            wv_r = ws.tile([128, 256], F32R, tag="w_r")
            wv_i = ws.tile([128, 256], F32R, tag="w_i")
            wv_n = ws.tile([128, 256], F32R, tag="w_n")
            nc.sync.dma_start(wv_r[:], wv_d[0, 128 * cj:128 * (cj + 1), :])
            nc.sync.dma_start(wv_i[:], wv_d[1, 128 * cj:128 * (cj + 1), :])
            nc.sync.dma_start(wv_n[:], wv_d[2, 128 * cj:128 * (cj + 1), :])
            st = cj == 0
            for kc in kcg:
                k0, szk = K_CHUNKS[kc]
                pr, pi = pps[kc]
                nc.tensor.matmul(pr[:szk, :256], _r(xnTr[:, cj, k0:k0 + szk]),
                                 _r(wv_r[:]), start=st, stop=False)
                nc.tensor.matmul(pr[:szk, :256], _r(xnTi[:, cj, k0:k0 + szk]),
                                 _r(wv_n[:]), start=False, stop=False)
                nc.tensor.matmul(pi[:szk, :256], _r(xnTr[:, cj, k0:k0 + szk]),
                                 _r(wv_i[:]), start=st, stop=False)
                nc.tensor.matmul(pi[:szk, :256], _r(xnTi[:, cj, k0:k0 + szk]),
                                 _r(wv_r[:]), start=False, stop=False)
        for kc in kcg:
            k0, szk = K_CHUNKS[kc]
            pr, pi = pps[kc]
            nc.tensor.matmul(pr[:szk, :256], _r(ones_row[:, :szk]), _r(bv_re[:]),
                             start=False, stop=True)
            nc.tensor.matmul(pi[:szk, :256], _r(ones_row[:, :szk]), _r(bv_im[:]),
                             start=False, stop=True)
            vr_v = pr[:szk, :256].rearrange("p (h d) -> p h d", h=4)
            vi_v = pi[:szk, :256].rearrange("p (h d) -> p h d", h=4)
            nc.scalar.copy(vpk[:szk, kc, :, 0:64], vr_v)
            nc.scalar.copy(vpk[:szk, kc, :, 64:128], vi_v)

    for s_ in range(3):
        nc.gpsimd.dma_start(wk_sb[:, :, s_, :],
                            wk_d[s_].rearrange("(j p) n -> p j n", p=128))
    for mc in range(5):
        t0, sz = MC[mc]
        _ln_sums(nc, work, st5, acc_re[:, mc], acc_im[:, mc], sz, mc)
    emit_qproj(*Q_CHUNKS[0])
    emit_qproj(*Q_CHUNKS[1])
    emit_qproj(*Q_CHUNKS[2])
    lnst[0] = _ln_stats(nc, stats, st5)
    for mc in range(5):
        emit_ln(mc)
        if mc > 2:
            emit_qproj(*Q_CHUNKS[mc])
        if mc == 2:
            emit_kproj(0)
        elif mc == 3:
            emit_vproj((0, 1, 2))
        elif mc == 4:
            emit_kproj(1)
            emit_vproj((3, 4))

    # phase-1/2 scratch pools release their SBUF before the attention
    # pools are opened (the tile framework time-shares the space)
    p1ctx.close()
    sm = ctx.enter_context(tc.tile_pool(name="sm", bufs=2))

    # =====================================================================
    # Phase 3: attention (S^T layout, f32r scores), softmax over |.|,
    # fused partial output projection per q-chunk.
    # =====================================================================
    wp_sb = big.tile([128, 3, 2, C], BF16, tag="wp")
    nc.sync.dma_start(wp_sb[:], wp_d.rearrange("s (j p) n -> p s j n", p=128))

    def emit_front_mm(q0, nq, hp):
        sbufA = sm.tile([128, 3, 2, 512], F16, tag="sbufA", bufs=2)
        sbufB = sm.tile([128, 2, 2, 512], F16, tag="sbufB", bufs=2)
        for kc in range(5):
            k0, szk = K_CHUNKS[kc]
            sbuf, kco = (sbufA, kc) if kc < 3 else (sbufB, kc - 3)
            for i in range(2):
                rs = slice(64 * i, 64 * (i + 1))
                sre = psum.tile([128, 512], F32, tag="bank")
                sim = psum.tile([128, 512], F32, tag="bank")
                nc.tensor.matmul(sre[:szk, :nq], _r(kTr[rs, hp, k0:k0 + szk]),
                                 qT_sb[rs, 0, hp, q0:q0 + nq], start=True,
                                 stop=False)
                nc.tensor.matmul(sim[:szk, :nq], _r(kTi[rs, hp, k0:k0 + szk]),
                                 qT_sb[rs, 0, hp, q0:q0 + nq], start=True,
                                 stop=False)
                nc.tensor.matmul(sre[:szk, :nq], _r(kTin[rs, hp, k0:k0 + szk]),
                                 qT_sb[rs, 1, hp, q0:q0 + nq], start=False,
                                 stop=True)
                nc.tensor.matmul(sim[:szk, :nq], _r(kTr[rs, hp, k0:k0 + szk]),
                                 qT_sb[rs, 1, hp, q0:q0 + nq], start=False,
                                 stop=True)
                t1 = sm.tile([128, 512], F16, tag="t1", bufs=2)
                t2 = sm.tile([128, 512], F16, tag="t2", bufs=2)
                nc.vector.tensor_copy(t1[:szk, :nq], sre[:szk, :nq])
                nc.vector.tensor_copy(t2[:szk, :nq], sim[:szk, :nq])
                u1 = sm.tile([128, 512], F16, tag="u1", bufs=1)
                u2 = sm.tile([128, 512], F16, tag="u2", bufs=1)
                nc.gpsimd.tensor_mul(u1[:szk, :nq], t1[:szk, :nq],
                                     t1[:szk, :nq])
                nc.gpsimd.tensor_mul(u2[:szk, :nq], t2[:szk, :nq],
                                     t2[:szk, :nq])
                nc.gpsimd.tensor_add(sbuf[:szk, kco, i, :nq], u1[:szk, :nq],
                                     u2[:szk, :nq])
        return sbufA, sbufB

    def emit_chain(q0, nq, hp, sbufA, sbufB):
        # |z| = exp(0.5 ln(|z|^2)); Ln/Exp/Square/Copy all live in the
        # natural_log_exp_and_others table, so no table reloads here.
        # constant shift keeps exp sums in f32 range; softmax is
        # shift-invariant so the result is exact
        ebuf = sm.tile([128, 5, 2, 512], BF16, tag="ebuf", bufs=4)
        for (sb_, kl, kn) in ((sbufA, 0, 3), (sbufB, 3, 2)):
            ks = slice(kl, kl + kn)
            ub = sm.tile([128, 3, 2, 512], F32, tag="ubuf", bufs=1)
            un = ub[:, :kn, :, :nq]
            nc.scalar.activation(un, sb_[:, :kn, :, :nq], AF.Ln)
            nc.scalar.activation(un, un, AF.Exp, scale=0.5)
            nc.scalar.activation(ebuf[:, ks, :, :nq], un, AF.Exp,
                                 bias=nbias[:])
        return ebuf

    def emit_backmm(q0, nq, hp, ebuf):
        op0 = psum.tile([128, 512], F32, tag="bank", name="op0")
        op1 = psum.tile([128, 512], F32, tag="bank", name="op1")
        dn = psum.tile([128, 512], F32, tag="bank", name="dn")
        for kc in range(5):
            k0, szk = K_CHUNKS[kc]
            for i in range(2):
                hh = 2 * hp + i
                opt = op0 if i == 0 else op1
                nc.tensor.matmul(opt[:, :nq], vpk[:szk, kc, hh, :],
                                 ebuf[:szk, kc, i, :nq], start=kc == 0,
                                 stop=kc == 4)
                nc.tensor.matmul(dn[32 * i:32 * i + 1, :nq], ones_col[:szk, :],
                                 ebuf[:szk, kc, i, :nq], start=kc == 0,
                                 stop=kc == 4, tile_position=(0, 32 * i))
        # tail stage 1: log-denominators on ACT (queued right after the
        # previous chain) + attn-out drained from PSUM by ACT.  One [33,*]
        # op covers both heads' rows (0 and 32); rows in between are
        # unread garbage.
        rh = stats.tile([33, 512], F32R, tag="lnd")
        nc.scalar.activation(rh[:, :nq], dn[0:33, :nq], AF.Ln)
        osbs = []
        for i, opt in enumerate((op0, op1)):
            osb = sm.tile([128, 512], F32, tag="osb", bufs=3)
            nc.scalar.copy(osb[:, :nq], opt[:, :nq])
            osbs.append(osb)
        return rh, osbs

    def emit_backtail(q0, nq, hp, rh, osbs):
        # tail stage 2 (one iteration later, so the PE never waits on ACT):
        # 1/dn broadcast + normalize
        otr = sm.tile([128, 512], BF16, tag="otr", bufs=4)
        oti = sm.tile([128, 512], BF16, tag="oti", bufs=4)
        nc.scalar.activation(rh[:, :nq], rh[:, :nq], AF.Exp, scale=-1.0)
        for i in range(2):
            rbp = psum.tile([128, 512], F32, tag="bank")
            nc.tensor.matmul(rbp[:, :nq], _r(ones2[32 * i:32 * i + 1, :]),
                             _r(rh[32 * i:32 * i + 1, :nq]),
                             start=True, stop=True)
            rb = sm.tile([128, 512], F32, tag="rb", bufs=1)
            nc.vector.tensor_copy(rb[:, :nq], rbp[:, :nq])
            osb = osbs[i]
            rs = slice(64 * i, 64 * (i + 1))
            nc.gpsimd.tensor_mul(otr[rs, :nq], osb[0:64, :nq], rb[0:64, :nq])
            nc.gpsimd.tensor_mul(oti[rs, :nq], osb[64:128, :nq], rb[64:128, :nq])
        return otr, oti

    def emit_proj(q0, nq, outs):
        for cc in range(4):
            cs = slice(128 * cc, 128 * (cc + 1))
            pr = psum.tile([128, 512], F32, tag="bank")
            pi = psum.tile([128, 512], F32, tag="bank")
            for hp in range(2):
                otr, oti = outs[hp]
                st = hp == 0
                sp = hp == 1
                nc.tensor.matmul(pr[:, :nq], wp_sb[:, 0, hp, cs],
                                 otr[:, :nq], start=st, stop=False)
                nc.tensor.matmul(pr[:, :nq], wp_sb[:, 2, hp, cs],
                                 oti[:, :nq], start=False, stop=sp)
                nc.tensor.matmul(pi[:, :nq], wp_sb[:, 1, hp, cs],
                                 otr[:, :nq], start=st, stop=False)
                nc.tensor.matmul(pi[:, :nq], wp_sb[:, 0, hp, cs],
                                 oti[:, :nq], start=False, stop=sp)
            o1 = sm.tile([128, 512], F16, tag="cp_r16")
            o2 = sm.tile([128, 512], F16, tag="cp_i16")
            nc.vector.tensor_copy(o1[:, :nq], pr[:, :nq])
            nc.vector.tensor_copy(o2[:, :nq], pi[:, :nq])
            nc.gpsimd.dma_start(outT_d[0, cs, q0:q0 + nq], o1[:, :nq])
            nc.sync.dma_start(outT_d[1, cs, q0:q0 + nq], o2[:, :nq])

    # software pipeline, 2 iterations deep, with a two-stage back tail:
    # iteration j runs the score matmuls of j, the attn@v of j-2, the
    # denominator stage-1 (ACT) of j-2, the normalize stage-2 of j-3 and
    # the output projection of ~j-4, so no engine ever waits on another
    # iteration's serial chain.
    fronts = []
    tails = []
    pending = {}
    ready = []
    def flush_back():
        q0_, nq_, hp_, ebA = fronts.pop(0)
        rh_, osbs_ = emit_backmm(q0_, nq_, hp_, ebA)
        tails.append((q0_, nq_, hp_, rh_, osbs_))
    def flush_tail():
        q0_, nq_, hp_, rhs_t, osbs = tails.pop(0)
        pending.setdefault(q0_, [None, None])[hp_] = \
            emit_backtail(q0_, nq_, hp_, rhs_t, osbs)
        if hp_ == 1:
            ready.append((q0_, nq_))
    def flush_proj():
        q_, n_ = ready.pop(0)
        emit_proj(q_, n_, pending.pop(q_))
    for (q0, nq) in Q_CHUNKS:
        for hp in range(2):
            sbufA, sbufB = emit_front_mm(q0, nq, hp)
            if len(fronts) >= 3:
                flush_back()
            if len(tails) >= 2:
                flush_tail()
            if len(ready) >= 2:
                flush_proj()
            fronts.append((q0, nq, hp, emit_chain(q0, nq, hp, sbufA, sbufB)))
    while fronts:
        flush_back()
        while len(tails) >= 2:
            flush_tail()
    while tails:
        flush_tail()
    while ready:
        flush_proj()

    ctx.close()


# =========================================================================
# Host side
# =========================================================================

def _f32(x):
    return np.ascontiguousarray(x, dtype=np.float32)


def _bf(x):
    return np.asarray(x, dtype=ml_dtypes.bfloat16)


def host_prep(x_re, x_im, Wq, Wkv, Wproj, bproj, sr_w, sr_b, gain, bias):
    x_re = np.asarray(x_re)
    x_im = np.asarray(x_im)
    Wq = np.asarray(Wq)
    Wkv = np.asarray(Wkv)
    Wproj = np.asarray(Wproj)
    sr_w = np.asarray(sr_w)
    sr_b = np.asarray(sr_b)
    gain = np.asarray(gain)
    bias = np.asarray(bias)

    Wkv_eff = gain[:, None] * Wkv
    bkv_full = bias @ Wkv
    Wc = sr_w.transpose(2, 3, 1, 0).reshape(4 * C, C)

    def planes3f(w):
        return np.stack([_f32(w.real), _f32(w.imag), _f32(-w.imag)])

    def planes3b(w):
        return np.stack([_bf(w.real), _bf(w.imag), _bf(-w.imag)])

    in_maps = []
    for core in range(8):
        b, g = core // 2, core % 2
        cols = slice(256 * g, 256 * (g + 1))
        wk_c = Wkv_eff[:, :C][:, cols] * SCALE
        wv_c = Wkv_eff[:, C:][:, cols]
        bk_c = bkv_full[:C][cols] * SCALE
        bv_c = bkv_full[C:][cols]
        xs_c = np.stack([x_re[b].T, x_im[b].T])  # [2, C, N]
        xsp = xs_c.reshape(2, C, HR, 2, HR, 2)
        xP = np.stack([xsp[:, :, :, p, :, q].reshape(2, C, NK)
                       for p in range(2) for q in range(2)], axis=1)
        m = {
            "xT": _f32(xs_c),
            "xP": _f32(xP.reshape(2, 4 * C, NK)),
            "wc": planes3f(Wc),
            "srb": np.stack([_f32(sr_b.real), _f32(sr_b.imag)]),
            "ones": np.ones((1, 512), np.float32),
            "wq": planes3f(Wq[:, cols]),
            "wk": planes3f(wk_c),
            "wv": planes3f(wv_c),
            "wp": planes3b(Wproj[256 * g:256 * (g + 1), :]),
            "bkv": np.stack([
                np.stack([_f32(bk_c.real), _f32(bv_c.real)]),
                np.stack([_f32(bk_c.imag), _f32(bv_c.imag)]),
            ]),
        }
        in_maps.append(m)
    return in_maps


_NC_CACHE = None


def _get_nc():
    global _NC_CACHE
    if _NC_CACHE is None:
        _NC_CACHE = build_nc()
    return _NC_CACHE


def kernel(x_re, x_im, Wq, Wkv, Wproj, bproj, sr_w, sr_b, gain, bias, H, W):
    from concourse.bass_utils import run_bass_kernel_spmd

    nc = _get_nc()
    in_maps = host_prep(x_re, x_im, Wq, Wkv, Wproj, bproj, sr_w, sr_b, gain, bias)
    res = run_bass_kernel_spmd(nc, in_maps, list(range(8)))
    bproj = np.asarray(bproj)
    out = np.zeros((B, N, C), dtype=np.complex64)
    for b in range(B):
        p0 = res.results[2 * b]["outT"].astype(np.float32)
        p1 = res.results[2 * b + 1]["outT"].astype(np.float32)
        acc = (p0[0] + p1[0]).T + 1j * (p0[1] + p1[1]).T
        out[b] = acc + bproj[None, :]
    return out

